# revision 9
# baseline (speedup 1.0000x reference)
"""CrossMerge kernel for trn2.

Math (per batch element):
    means_i = mean over C of g_i              (4, H, W)
    logits  = w_proj @ means + b_proj         (4, H, W)
    w       = softmax(logits, axis=0)         (4, H, W)
    out     = sum_i g_i * w_i                 (C, H, W)

Sharding: data-parallel over batch B=8 across 8 cores; weights replicated;
no cross-device communication.

Per-core layout: the 4 grids are host-stacked into gall (4, 256, 9216);
C=256 is split into 2 partition-chunks of 128.  Spatial axis tiled into
9 DMA tiles of 1024 cols (one 4 MB load + one 1 MB store each), each
split into 2 compute slices of 512 cols (fp32 PSUM bank width).

v2 design (PE was the bottleneck at 167us vs the 131us DMA roofline):
the product-accumulation identity matmuls (144 x ~680ns + ldweights)
are GONE.  Products are written as fp16 and the 4-way sum runs as an
fp16 add tree on DVE/gpsimd (fp16 gets the DVE 2x/4x fast modes), with
the final add writing fp32 straight into the output tile (no PSUM->SBUF
copy either).  Broadcast + denominator matmuls run with fp16 moving
data (1 cycle/row instead of fp32-HIGH).  Remaining PE work: the
irreducible fp32r logits pass + fp16 denom/broadcast ~= 106us < DMA.

Per 512-col slice j:
  PE  : 8 mm  logits L(4,512) += ws_i(128,4)^T g_ic(128,512)   [fp32r]
        1 mm  S4(4,512) = ones4x4^T E       (softmax denominator, fp16)
        4 mm  wb_i(128,512) = row-select broadcast of W4[i]     [fp16]
  ACT : E = exp(L + b) -> fp16;  4x copy wb PSUM->SBUF fp32 (wbs)
  DVE : R4 = reciprocal_approx_fast(S4); W4 = E * R4 -> fp16
        products p = g * wbs -> fp16 (grids 0,1 + grid2/c0)
        q01/q23 pair adds (fp16, fast mode); final add c=1 -> fp32 ot
  POOL: products for grid3 + grid2/c1; final add c=0 -> fp32 ot

Emission is software-pipelined one d-tile deep: block d emits the
narrow chain of d (dma, logits, exp, denom, recip, W4) followed by the
wide stage of d-1 (broadcast, wbs copies, products, adds, store), so no
engine's in-order stream ever waits on a same-block cross-engine round
trip.

All narrow softmax tiles keep matmul outputs at PSUM base partition 0
(reciprocal custom DVE op malfunctions at a nonzero base partition,
HW-verified).

Codegen constraint honored throughout: TRN2 instructions support a
single sync wait; Bacc's generate_event_semaphores pass splits the rest.
"""

import os
import sys
from contextlib import ExitStack

import numpy as np

try:
    import concourse.bass as bass
except ImportError:  # fresh grading dir: concourse lives in the container repo
    sys.path.insert(0, "/opt/trn_rl_repo")
    import concourse.bass as bass

import concourse.tile as tile
from concourse import bacc, mybir
from concourse.bass_utils import run_bass_kernel_spmd

B, C, H, W = 8, 256, 96, 96
HW = H * W  # 9216
NCORES = 8
CPB = C // 128  # 2 partition chunks per core
DCOLS = 1024  # columns per DMA tile
JCOLS = 512  # columns per compute slice (= fp32 PSUM bank)
NDMA = HW // DCOLS  # 9
NJ = DCOLS // JCOLS  # 2

F32 = mybir.dt.float32
F32R = mybir.dt.float32r
F16 = mybir.dt.float16
AF = mybir.ActivationFunctionType


def dve_takes(c, i):
    """Product (chunk c, grid i) on DVE? Rest go to gpsimd.
    gpsimd (Q7 software ucode) only ever sees fp32 operands — fp16 on the
    Pool engine hangs the ucode dispatch (HW-observed EXEC_UNIT fault)."""
    return i in (0, 1) or (i == 2 and c == 0)


_CACHE = {}


def build_program():
    nc = bacc.Bacc("TRN2", debug=False, num_devices=NCORES)

    gall_d = nc.dram_tensor("gall", [4, C, HW], F32R, kind="ExternalInput").ap()
    # fp32 constants: cols 0-15 ws (w_proj/C replicated down partitions),
    # col 16 bias (rows 0-3)
    cb_d = nc.dram_tensor("cblob", [128, 17], F32R, kind="ExternalInput").ap()
    # fp16 constants: cols 0-511 selmat, cols 512-515 ones4x4 (rows 0-3)
    ch_d = nc.dram_tensor("cblob16", [128, 516], F16, kind="ExternalInput").ap()
    out = nc.dram_tensor("out", [C, HW], F32, kind="ExternalOutput").ap()

    with tile.TileContext(nc) as tc, ExitStack() as ctx:
        const = ctx.enter_context(tc.tile_pool(name="const", bufs=1))
        gin = ctx.enter_context(tc.tile_pool(name="gin", bufs=3))
        outp = ctx.enter_context(tc.tile_pool(name="outp", bufs=2))
        narrow = ctx.enter_context(tc.tile_pool(name="narrow", bufs=3))
        wbsb = ctx.enter_context(tc.tile_pool(name="wbsb", bufs=3))
        prod = ctx.enter_context(tc.tile_pool(name="prod", bufs=12))
        qpool = ctx.enter_context(tc.tile_pool(name="qpool", bufs=8))
        ps_smx = ctx.enter_context(tc.tile_pool(name="psmx", bufs=2, space="PSUM"))
        ps_S4 = ctx.enter_context(tc.tile_pool(name="psS4", bufs=2, space="PSUM"))
        ps_Wb = ctx.enter_context(tc.tile_pool(name="psWb", bufs=1, space="PSUM"))

        # constants -> SBUF (two small DMAs)
        cb = const.tile([128, 17], F32R)
        nc.sync.dma_start(out=cb[:], in_=cb_d)
        ch = const.tile([128, 516], F16)
        nc.sync.dma_start(out=ch[:], in_=ch_d)
        ws = cb[:, 0:16]
        bv = cb[0:4, 16:17].bitcast(F32)
        selmat = ch[0:4, 0:512]
        ones4x4 = ch[0:4, 512:516]

        # Warmup matmul: absorbs the const-blob DMA wait on the PE clock.
        warm = ps_S4.tile([4, 16], F32, tag="S4")
        nc.tensor.matmul(warm[:], lhsT=ws[:, 0:4], rhs=ws, start=True, stop=True)

        def narrow_stage(gat):
            """logits + exp for both j slices, then denom/recip/W4.
            Returns per-j W4 (fp16, SBUF)."""
            Ls, Es = [], []
            for j in range(NJ):
                x0 = j * JCOLS
                smx = ps_smx.tile([128, JCOLS], F32, tag="smx")
                L = smx[0:4, :]
                k = 0
                for i in range(4):
                    for c in range(CPB):
                        nc.tensor.matmul(
                            L,
                            lhsT=ws[:, 4 * i : 4 * i + 4],
                            rhs=gat[:, i, c, x0 : x0 + JCOLS],
                            start=(k == 0),
                            stop=(k == 7),
                        )
                        k += 1
                E = narrow.tile([4, JCOLS], F16, tag="E")
                nc.scalar.activation(E[:], L, AF.Exp, bias=bv, scale=1.0)
                Ls.append(L)
                Es.append(E)
            S4s = []
            for j in range(NJ):
                S4 = ps_S4.tile([4, JCOLS], F32, tag="S4")
                nc.tensor.matmul(S4[:], lhsT=ones4x4, rhs=Es[j][:], start=True, stop=True)
                S4s.append(S4)
            W4s = []
            for j in range(NJ):
                R4 = narrow.tile([4, JCOLS], F32, tag="R4")
                nc.vector.reciprocal_approx_fast(R4[:], S4s[j][:])
                # consumed by the NEXT block's broadcast: needs depth so the
                # WAR dep never lands in DVE's in-order stream mid-pipeline
                W4 = narrow.tile([4, JCOLS], F16, tag="W4", bufs=5)
                nc.vector.tensor_mul(W4[:], Es[j][:], R4[:])
                W4s.append(W4)
            return W4s

        def wide_stage(prev):
            """broadcast + products + add tree + store for iter d-1."""
            if prev is None:
                return
            d, gat, ot, W4s = prev
            # broadcast weights to 128 partitions (PE, fp16 moving) and
            # stage them in SBUF via ACT (frees the PSUM bank quickly,
            # gives gpsimd SBUF operands).
            wbs = {}
            for j in range(NJ):
                for i in range(4):
                    Wbp = ps_Wb.tile([128, JCOLS], F32, tag=f"wb{i}")
                    nc.tensor.matmul(
                        Wbp[:],
                        lhsT=selmat[:, 128 * i : 128 * (i + 1)],
                        rhs=W4s[j][:],
                        start=True,
                        stop=True,
                    )
                    Wb = wbsb.tile([128, JCOLS], F32, tag=f"wbs{i}")
                    nc.scalar.copy(Wb[:], Wbp[:])
                    wbs[(j, i)] = Wb
            # products: DVE -> fp16, gpsimd -> fp32 (Q7 ucode is fp32-only)
            p = {}
            for j in range(NJ):
                x0 = j * JCOLS
                for c in range(CPB):
                    for i in range(4):
                        on_dve = dve_takes(c, i)
                        pt = prod.tile(
                            [128, JCOLS],
                            F16 if on_dve else F32,
                            tag="p16" if on_dve else "p32",
                            bufs=12 if on_dve else 8,
                        )
                        gslice = gat[:, i, c, x0 : x0 + JCOLS].bitcast(F32)
                        eng = nc.vector if on_dve else nc.gpsimd
                        eng.tensor_mul(pt[:], gslice, wbs[(j, i)][:])
                        p[(j, c, i)] = pt
            # pair-add tree; q01 is all-fp16 on DVE (fast mode), the grid-2/3
            # pair sits on whichever engine owns its inputs' dtypes
            for j in range(NJ):
                x0 = j * JCOLS
                for c in range(CPB):
                    q01 = qpool.tile([128, JCOLS], F16, tag="q16")
                    nc.vector.tensor_add(q01[:], p[(j, c, 0)][:], p[(j, c, 1)][:])
                    if c == 0:
                        # p2 fp16 (DVE) + p3 fp32 -> DVE mixed add
                        q23 = qpool.tile([128, JCOLS], F16, tag="q16")
                        nc.vector.tensor_add(q23[:], p[(j, c, 2)][:], p[(j, c, 3)][:])
                    else:
                        # both fp32 (gpsimd-made) -> gpsimd fp32 add
                        q23 = qpool.tile([128, JCOLS], F32, tag="q32", bufs=3)
                        nc.gpsimd.tensor_add(q23[:], p[(j, c, 2)][:], p[(j, c, 3)][:])
                    nc.vector.tensor_add(ot[:, c, x0 : x0 + JCOLS], q01[:], q23[:])
            n0 = d * DCOLS
            nc.sync.dma_start(
                out=out[:, n0 : n0 + DCOLS].rearrange("(c p) n -> p c n", c=CPB),
                in_=ot[:],
            )

        prev = None
        for d in range(NDMA):
            n0 = d * DCOLS
            gat = gin.tile([128, 4, CPB, DCOLS], F32R, tag="gall")
            nc.sync.dma_start(
                out=gat[:],
                in_=gall_d[:, :, n0 : n0 + DCOLS].rearrange(
                    "i (c p) n -> p i c n", c=CPB
                ),
            )
            ot = outp.tile([128, CPB, DCOLS], F32, tag="ot")
            W4s = narrow_stage(gat)
            wide_stage(prev)
            prev = (d, gat, ot, W4s)
        wide_stage(prev)

    nc.compile()
    return nc


def _get_program():
    if "nc" not in _CACHE:
        _CACHE["nc"] = build_program()
    return _CACHE["nc"]


def make_cblobs(w_proj, b_proj):
    w = np.asarray(w_proj, dtype=np.float32)
    b = np.asarray(b_proj, dtype=np.float32)
    ws = np.empty((128, 16), dtype=np.float32)
    for i in range(4):
        for o in range(4):
            ws[:, 4 * i + o] = w[o, i] / C
    cblob = np.zeros((128, 17), dtype=np.float32)
    cblob[:, 0:16] = ws
    cblob[0:4, 16] = b
    ch = np.zeros((128, 516), dtype=np.float16)
    ch[0:4, 0:512] = np.repeat(np.eye(4, dtype=np.float16), 128, axis=1)
    ch[0:4, 512:516] = 1.0
    return cblob, ch


LAST_RESULT = None


def kernel(g0, g1, g2, g3, w_proj, b_proj):
    global LAST_RESULT
    nc = _get_program()

    cblob, ch = make_cblobs(w_proj, b_proj)

    gall = np.stack(
        [np.asarray(x, dtype=np.float32).reshape(B, C, HW) for x in (g0, g1, g2, g3)],
        axis=1,
    )  # (B, 4, C, HW)
    in_maps = []
    for bi in range(NCORES):
        m = {"gall": np.ascontiguousarray(gall[bi]), "cblob": cblob, "cblob16": ch}
        in_maps.append(m)

    res = run_bass_kernel_spmd(
        nc,
        in_maps,
        list(range(NCORES)),
        trace=bool(int(os.environ.get("CM_TRACE", "0"))),
        tmpdir=os.environ.get("CM_TRACE_DIR") or None,
    )
    LAST_RESULT = res
    out_full = np.stack(
        [res.results[bi]["out"].reshape(C, H, W) for bi in range(NCORES)], axis=0
    )
    return out_full
